# revision 26
# baseline (speedup 1.0000x reference)
"""RoFormer self-attention (LN + fused QKV + interleaved RoPE + SDPA) on 8 trn2 cores.

Sharding: core c -> batch b = c//2, head-group g = c%2 (8 of 16 heads).
Each core computes LN + QKV for its batch, RoPE, and full non-causal
attention for its 8 heads, writing the [2048, 512] slice
out[b, :, 512g:512(g+1)]. No collectives.

Numerics: all matmuls in bf16 with fp32 PSUM accumulation (rel err ~4e-3).
Softmax skips max-subtraction; exp work is split between the Activation
engine (native Exp) and DVE (pow(e^scale, x) with a broadcast base).
QKV q/k biases are added on DVE/Pool as fused bias+copy ops out of PSUM;
the v bias is folded into the context output (ctx(v+bv) = ctx(v) + bv).
Softmax denominator comes from an appended ones-column on V and is applied
with a post-transpose `divide` on the Pool engine.
"""

import numpy as np

import concourse.bass as bass
import concourse.mybir as mybir
import concourse.tile as tile
from concourse import bacc
from concourse.masks import make_identity
from concourse.bass_utils import run_bass_kernel_spmd

F32 = mybir.dt.float32
F32R = mybir.dt.float32r
BF16 = mybir.dt.bfloat16
AX = mybir.AluOpType
ACT = mybir.ActivationFunctionType

B, S, H = 4, 2048, 1024
NH, HD = 16, 64
LN_EPS = 1e-12
N_CORES = 8
HPC = NH // 2          # 8 heads per core
WCOLS = 3 * HPC * HD   # 1536
TOKCH = S // 128       # 16 token chunks
SCALE = 1.0 / np.sqrt(HD)
QW = 512               # q columns per attention unit
NQQ = S // QW          # 4 q-quarters
CTX_DELAY = 24         # iterations between score matmul and ctx matmul
EXP_MOD = 5            # exp engine pattern period
EXP_ACT = 3            # iters per period routed to ACT (rest: DVE copy + Pool pow)
P2BUFS = 27            # P tile ring depth

_CACHE = {}


def _bcast(ap, n, axis=1):
    """Insert a stride-0 broadcast dim of size n at `axis` of an AP."""
    new = [list(p) for p in ap.ap]
    new.insert(axis, [0, n])
    return bass.AP(tensor=ap.tensor, offset=ap.offset, ap=new)


def _build_program():
    nc = bacc.Bacc("TRN2", target_bir_lowering=False)

    hid_d = nc.dram_tensor("hid", [S, H], F32, kind="ExternalInput")
    w_d = nc.dram_tensor("w", [H, WCOLS], BF16, kind="ExternalInput")
    sin_d = nc.dram_tensor("sintab", [S, HD], F32, kind="ExternalInput")
    cos_d = nc.dram_tensor("costab", [S, HD], F32, kind="ExternalInput")
    bq_d = nc.dram_tensor("bq", [128, 512], F32, kind="ExternalInput")
    bk_d = nc.dram_tensor("bk", [128, 512], F32, kind="ExternalInput")
    bv_d = nc.dram_tensor("bv", [128, 512], F32, kind="ExternalInput")
    out_d = nc.dram_tensor("out", [S, HPC * HD], F32, kind="ExternalOutput")

    with tile.TileContext(nc) as tc:
        with tc.tile_pool(name="const", bufs=1) as const, \
             tc.tile_pool(name="store", bufs=1) as store:
            idb_s = const.tile([128, 128], BF16)
            make_identity(nc, idb_s)
            sin_s = const.tile([128, TOKCH, HD], F32)
            cos_s = const.tile([128, TOKCH, HD], F32)
            bq_s = const.tile([128, HPC, HD], F32)
            bk_s = const.tile([128, HPC, HD], F32)
            bv_s = const.tile([128, HPC * HD], F32)
            base_s = const.tile([128, QW], F32)
            nc.vector.memset(base_s, float(np.exp(SCALE)))
            eps_s = const.tile([128, 1], F32)
            nc.vector.memset(eps_s, LN_EPS)
            expwarm = const.tile([128, 1], F32)
            nc.scalar.activation(expwarm, eps_s, ACT.Exp)
            nhalf_s = const.tile([128, 1], F32)
            nc.vector.memset(nhalf_s, -0.5)

            # Transposed per-head q/k: head h lives at partitions (h%2)*64,
            # pair index h//2:  [128, 4, TOKCH, 128]  (= [64, tokch*128]/head)
            qT = store.tile([128, HPC // 2, TOKCH, 128], BF16)
            kT = store.tile([128, HPC // 2, TOKCH, 128], BF16)
            # v with appended ones column: [tok, chunk, head, 65]
            vA = store.tile([128, TOKCH, HPC, HD + 1], BF16)
            nc.vector.memset(vA[:, :, :, HD:HD + 1], 1.0)

            # ---------------- Phase 1: LN + QKV + RoPE + transposes ----------
            with tc.tile_pool(name="wpool", bufs=1) as wpool, \
                 tc.tile_pool(name="p1", bufs=2) as p1, \
                 tc.tile_pool(name="p1h", bufs=7) as p1h, \
                 tc.tile_pool(name="p1n", bufs=3) as p1n, \
                 tc.tile_pool(name="p1s", bufs=4) as p1s, \
                 tc.tile_pool(name="trq", bufs=4, space="PSUM") as trq, \
                 tc.tile_pool(name="qkvp", bufs=1, space="PSUM") as qkvp:
                w_r = w_d.rearrange("(a p) n -> p a n", p=128)
                w_s = []
                for hc in range(H // 128):
                    wt = wpool.tile([128, WCOLS], BF16, tag=f"w{hc}", name=f"w{hc}")
                    w_s.append(wt)
                ht_tiles = {}
                ht = p1h.tile([128, H], F32, tag="ht")
                nc.sync.dma_start(out=ht, in_=hid_d[0:128, :])
                ht_tiles[0] = ht
                # PE p-state warmup while DMAs land (~3us of continuous busy)
                for wu in range(90):
                    ptw = trq.tile([128, 4, 128], BF16, tag="pt4",
                                   padded_shape=[128, 4, 256])
                    nc.tensor.transpose(ptw[:, 0, :], idb_s, idb_s)
                # w has priority: chunk 0's QKV needs the full weight load
                for hc in range(H // 128):
                    nc.sync.dma_start(out=w_s[hc], in_=w_r[:, hc, :])
                for tpre in range(1, 6):
                    htn = p1h.tile([128, H], F32, tag="ht", name=f"ht{tpre}")
                    nc.sync.dma_start(out=htn, in_=hid_d[tpre * 128:(tpre + 1) * 128, :])
                    ht_tiles[tpre] = htn
                nc.sync.dma_start(out=bq_s.rearrange("p h d -> p (h d)"), in_=bq_d[:, :])
                nc.sync.dma_start(out=bk_s.rearrange("p h d -> p (h d)"), in_=bk_d[:, :])
                nc.sync.dma_start(out=sin_s, in_=sin_d.rearrange("(t p) d -> p t d", p=128))
                nc.sync.dma_start(out=cos_s, in_=cos_d.rearrange("(t p) d -> p t d", p=128))
                nc.sync.dma_start(out=bv_s, in_=bv_d[:, :])

                def rope_pre(t, pq):
                    # GPSIMD cannot touch PSUM: DVE does both fused
                    # bias-add + PSUM->SBUF copies, emitted early so the
                    # Pool-side rope chain can start promptly.
                    outs = []
                    for nch in range(2):
                        q0 = p1.tile([128, HPC, HD], F32, tag=f"q0{nch}", name="q0")
                        nc.vector.tensor_tensor(out=q0,
                                          in0=pq[nch].rearrange("p (h d) -> p h d", d=HD),
                                          in1=bq_s if nch == 0 else bk_s, op=AX.add)
                        outs.append(q0)
                    return outs

                def rope_block(t, q0k0):
                    sin_t = sin_s[:, t, :]
                    cos_t = cos_s[:, t, :]
                    for nch in range(2):
                        eng = nc.vector if nch == 0 else nc.gpsimd
                        q0 = q0k0[nch]
                        rp = p1.tile([128, HPC, HD], F32, tag=f"rp{nch}")
                        eng.tensor_tensor(out=rp[:, :, 0::2], in0=q0[:, :, 1::2],
                                          in1=_bcast(sin_t[:, 0::2], HPC), op=AX.mult)
                        eng.tensor_tensor(out=rp[:, :, 1::2], in0=q0[:, :, 0::2],
                                          in1=_bcast(sin_t[:, 1::2], HPC), op=AX.mult)
                        qf = p1.tile([128, HPC, HD], BF16, tag=f"qf{nch}")
                        eng.tensor_tensor(out=qf, in0=q0, in1=_bcast(cos_t, HPC),
                                          op=AX.mult)
                        eng.tensor_tensor(out=qf, in0=qf, in1=rp, op=AX.add)

                        dst = qT if nch == 0 else kT
                        pt4 = trq.tile([128, HPC // 2, 128], BF16, tag="pt4",
                                       padded_shape=[128, HPC // 2, 256])
                        qfv = qf.rearrange("p h d -> p (h d)")
                        for j in range(HPC // 2):
                            nc.tensor.transpose(pt4[:, j, :],
                                                qfv[:, j * 128:(j + 1) * 128], idb_s)
                        if nch == 0:
                            nc.vector.tensor_copy(dst[:, :, t, :], pt4)
                        else:
                            nc.scalar.copy(dst[:, :, t, :], pt4)

                pq_of = {}
                pre_of = {}
                hn_of = {}

                def ln_block(t):
                    ht = ht_tiles.pop(t)
                    st6 = p1s.tile([128, 2, 6], F32, tag="st6")
                    for half in range(2):
                        nc.vector.bn_stats(out=st6[:, half, :],
                                           in_=ht[:, half * 512:(half + 1) * 512])
                    mv = p1s.tile([128, 2], F32, tag="mv")
                    nc.vector.bn_aggr(out=mv, in_=st6)
                    vpe = p1s.tile([128, 1], F32, tag="vpe")
                    nc.gpsimd.tensor_scalar(out=vpe, in0=mv[:, 1:2], scalar1=LN_EPS,
                                            scalar2=None, op0=AX.add)
                    rstd = p1s.tile([128, 1], F32, tag="rstd")
                    nc.gpsimd.tensor_tensor(out=rstd, in0=vpe, in1=nhalf_s, op=AX.pow)
                    nmr = p1s.tile([128, 1], F32, tag="nmr")
                    nc.gpsimd.tensor_scalar(out=nmr, in0=mv[:, 0:1], scalar1=rstd,
                                            scalar2=-1.0, op0=AX.mult, op1=AX.mult)
                    hn = p1n.tile([128, H], BF16, tag="hn")
                    nc.scalar.activation(hn, ht, ACT.Identity, bias=nmr, scale=rstd)
                    return hn

                hn_of[0] = ln_block(0)
                for t in range(TOKCH):
                    if t + 6 < TOKCH:
                        htn = p1h.tile([128, H], F32, tag="ht")
                        nc.sync.dma_start(out=htn, in_=hid_d[(t + 6) * 128:(t + 7) * 128, :])
                        ht_tiles[t + 6] = htn
                    # early: free last iterations' QKV PSUM slots
                    if t - 1 in pq_of:
                        nc.scalar.copy(vA[:, t - 1, :, 0:HD],
                                       pq_of[t - 1][2].rearrange("p (h d) -> p h d", d=HD))
                    if t - 2 in pq_of:
                        pre_of[t - 2] = rope_pre(t - 2, pq_of.pop(t - 2))
                    if t + 1 < TOKCH:
                        hn_of[t + 1] = ln_block(t + 1)
                    hn = hn_of.pop(t)

                    # transpose hn -> hT [hch, tok]
                    hT = p1.tile([128, H // 128, 128], BF16, tag="hT")
                    for g in range(2):
                        ptg = trq.tile([128, 4, 128], BF16, tag="pt4",
                                       padded_shape=[128, 4, 256])
                        for hc in range(4):
                            nc.tensor.transpose(ptg[:, hc, :],
                                                hn[:, (g * 4 + hc) * 128:(g * 4 + hc + 1) * 128],
                                                idb_s)
                        nc.vector.tensor_copy(hT[:, g * 4:(g + 1) * 4, :], ptg)

                    # QKV: out[tok, n] accumulated over h-chunks. Chunk 0 is
                    # paced by the weight DMA: go hc-major with keep-warm
                    # transposes stuffed between chunks so the PE p-state
                    # ramp survives the DMA waits.
                    pq = []
                    for nch in range(3):
                        bufs = 2 if nch == 0 else 1
                        pp = qkvp.tile([128, 512], F32, tag=f"qkv{nch}", bufs=bufs)
                        pq.append(pp)
                    if t == 0:
                        for hc in range(H // 128):
                            for nch in range(3):
                                nc.tensor.matmul(pq[nch], lhsT=hT[:, hc, :],
                                                 rhs=w_s[hc][:, nch * 512:(nch + 1) * 512],
                                                 start=(hc == 0), stop=(hc == H // 128 - 1))
                            if hc < H // 128 - 1:
                                for wu in range(8):
                                    ptw = trq.tile([128, 4, 128], BF16, tag="pt4",
                                                   padded_shape=[128, 4, 256])
                                    nc.tensor.transpose(ptw[:, 0, :], idb_s, idb_s)
                    else:
                        for nch in range(3):
                            for hc in range(H // 128):
                                nc.tensor.matmul(pq[nch], lhsT=hT[:, hc, :],
                                                 rhs=w_s[hc][:, nch * 512:(nch + 1) * 512],
                                                 start=(hc == 0), stop=(hc == H // 128 - 1))
                    pq_of[t] = pq

                    # tighten the tail: last chunk ropes at delay 0/1
                    if t == TOKCH - 1:
                        nc.scalar.copy(vA[:, t, :, 0:HD],
                                       pq[2].rearrange("p (h d) -> p h d", d=HD))
                        for t_ in (t - 1, t):
                            if t_ in pq_of:
                                pre_of[t_] = rope_pre(t_, pq_of.pop(t_))

                    # late: rope math + transposes for t-2
                    if t - 2 in pre_of:
                        rope_block(t - 2, pre_of.pop(t - 2))

                # drain remaining rope math
                for t_ in sorted(pre_of):
                    rope_block(t_, pre_of.pop(t_))

            # ---------------- Phase 2: attention per (head, q-quarter) -------
            with tc.tile_pool(name="p2", bufs=P2BUFS) as p2, \
                 tc.tile_pool(name="p2r", bufs=4) as p2r, \
                 tc.tile_pool(name="p2c", bufs=2) as p2c, \
                 tc.tile_pool(name="p2o", bufs=2) as p2o, \
                 tc.tile_pool(name="ctxp", bufs=2, space="PSUM") as ctxp, \
                 tc.tile_pool(name="ctxo", bufs=2, space="PSUM") as ctxo, \
                 tc.tile_pool(name="stp", bufs=4, space="PSUM") as stp:
                units = [(h, qq) for h in range(HPC) for qq in range(NQQ)]
                ctx_of = {}

                def emit_score(u, kc):
                    h, qq = u
                    po = (h % 2) * 64
                    pr = h // 2
                    sp = stp.tile([128, QW], F32, tag="st")
                    nc.tensor.matmul(
                        sp,
                        lhsT=kT[po:po + 64, pr, kc, :],
                        rhs=qT[po:po + 64, pr, qq * 4:(qq + 1) * 4, :],
                        start=True, stop=True)
                    return sp

                def emit_exp(i, sp):
                    # DVE/Pool have no exp; Pool has pow but cannot read
                    # PSUM. Route 3/5 through ACT directly and 2/5 through
                    # a DVE PSUM->SBUF copy followed by Pool pow(e^s, x).
                    P = p2.tile([128, QW], BF16, tag="P")
                    if i % EXP_MOD < EXP_ACT:
                        nc.scalar.activation(P, sp, ACT.Exp, scale=SCALE)
                    else:
                        spb = p2.tile([128, QW], F32, tag="spb", bufs=4)
                        nc.vector.tensor_copy(spb, sp)
                        nc.gpsimd.tensor_tensor(out=P, in0=base_s, in1=spb,
                                                op=AX.pow)
                    return P

                def emit_ctx(u, kc, P):
                    h, qq = u
                    if kc == 0:
                        ctx_of[u] = ctxp.tile([HD + 1, QW], F32, tag="ctx", name="cp")
                    cp = ctx_of[u]
                    nc.tensor.matmul(cp, lhsT=vA[:, kc, h, :], rhs=P,
                                     start=(kc == 0), stop=(kc == TOKCH - 1))
                    if kc == TOKCH - 1:
                        emit_tail(u, cp)

                def emit_tail(u, cp):
                    h, qq = u
                    ctxs = p2c.tile([HD + 1, QW], BF16, tag="ctxs")
                    nc.scalar.copy(ctxs, cp)
                    outt = p2o.tile([128, QW // 128, HD], F32, tag="outt")
                    for tc_ in range(QW // 128):
                        co = ctxo.tile([128, HD + 1], BF16, tag="co",
                                       padded_shape=[128, 1024])
                        nc.tensor.transpose(co, ctxs[:, tc_ * 128:(tc_ + 1) * 128],
                                            idb_s[0:HD + 1, 0:HD + 1])
                        rec = p2r.tile([128, 1], F32, tag="rec")
                        nc.vector.reciprocal(rec, co[:, HD:HD + 1])
                        nc.vector.tensor_scalar(out=outt[:, tc_, :], in0=co[:, 0:HD],
                                                scalar1=rec, scalar2=None,
                                                op0=AX.mult)
                    # fold in the v bias: ctx(v + bv) = ctx(v) + bv
                    nc.gpsimd.tensor_tensor(
                        out=outt, in0=outt,
                        in1=_bcast(bv_s[:, h * HD:(h + 1) * HD], QW // 128),
                        op=AX.add)
                    dst = out_d[qq * QW:(qq + 1) * QW, h * HD:(h + 1) * HD]
                    nc.sync.dma_start(
                        out=dst.rearrange("(c p) d -> p c d", p=128), in_=outt)

                stream = [(u, kc) for u in units for kc in range(TOKCH)]
                pend = []  # [(u, kc, P)]
                for i, (u, kc) in enumerate(stream):
                    sp = emit_score(u, kc)
                    P = emit_exp(i, sp)
                    pend.append((u, kc, P))
                    if len(pend) > CTX_DELAY:
                        emit_ctx(*pend.pop(0))
                for item in pend:
                    emit_ctx(*item)

    nc.compile()
    return nc


def _host_inputs(hidden_states, sinusoidal_pos, ln_weight, ln_bias, w_qkv, b_qkv):
    """Build the per-core input maps."""
    import ml_dtypes

    hidden_states = np.ascontiguousarray(hidden_states, dtype=np.float32)
    w_qkv = np.asarray(w_qkv, dtype=np.float32)
    b_qkv = np.asarray(b_qkv, dtype=np.float32)
    ln_weight = np.asarray(ln_weight, dtype=np.float32)
    ln_bias = np.asarray(ln_bias, dtype=np.float32)
    sp = np.asarray(sinusoidal_pos, dtype=np.float32).reshape(S, HD)

    # Fold LayerNorm affine params into the projection.
    w_eff = ln_weight[:, None] * w_qkv          # [H, 3H]
    b_eff = b_qkv + ln_bias @ w_qkv             # [3H]

    sin = sp[:, :HD // 2]
    cos = sp[:, HD // 2:]
    sin_pos = np.repeat(sin, 2, axis=1)          # [S, 64], col 2i = 2i+1 = sin_i
    cos_pos = np.repeat(cos, 2, axis=1)
    sgn = np.ones((1, HD), np.float32)
    sgn[0, 0::2] = -1.0
    sin_signed = (sin_pos * sgn).astype(np.float32)  # col 2i = -sin_i, 2i+1 = sin_i

    in_maps = []
    for c in range(N_CORES):
        b = c // 2
        g = c % 2
        cols = np.concatenate([
            np.arange(g * 512, (g + 1) * 512),
            1024 + np.arange(g * 512, (g + 1) * 512),
            2048 + np.arange(g * 512, (g + 1) * 512),
        ])
        bias = b_eff[cols]
        in_maps.append({
            "hid": hidden_states[b],
            "w": np.ascontiguousarray(w_eff[:, cols]).astype(ml_dtypes.bfloat16),
            "bq": np.ascontiguousarray(
                np.broadcast_to(bias[None, 0:512], (128, 512))),
            "bk": np.ascontiguousarray(
                np.broadcast_to(bias[None, 512:1024], (128, 512))),
            "bv": np.ascontiguousarray(
                np.broadcast_to(bias[None, 1024:1536], (128, 512))),
            "sintab": sin_signed,
            "costab": cos_pos,
        })
    return in_maps


def _run(trace=False, **inputs):
    if "nc" not in _CACHE:
        _CACHE["nc"] = _build_program()
    nc = _CACHE["nc"]
    in_maps = _host_inputs(**inputs)
    res = run_bass_kernel_spmd(nc, in_maps, core_ids=list(range(N_CORES)),
                               trace=trace)
    out = np.empty((B, S, H), np.float32)
    for c in range(N_CORES):
        b = c // 2
        g = c % 2
        out[b, :, g * 512:(g + 1) * 512] = res.results[c]["out"]
    return out, res


def kernel(**inputs):
    out, _ = _run(trace=False, **inputs)
    return out


def kernel_traced(**inputs):
    return _run(trace=True, **inputs)


# revision 27
# speedup vs baseline: 1.0069x; 1.0069x over previous
"""RoFormer self-attention (LN + fused QKV + interleaved RoPE + SDPA) on 8 trn2 cores.

Sharding: core c -> batch b = c//2, head-group g = c%2 (8 of 16 heads).
Each core computes LN + QKV for its batch, RoPE, and full non-causal
attention for its 8 heads, writing the [2048, 512] slice
out[b, :, 512g:512(g+1)]. No collectives.

Numerics: all matmuls in bf16 with fp32 PSUM accumulation (rel err ~4e-3).
Softmax skips max-subtraction; exp work is split between the Activation
engine (native Exp) and DVE (pow(e^scale, x) with a broadcast base).
QKV q/k biases are added on DVE/Pool as fused bias+copy ops out of PSUM;
the v bias is folded into the context output (ctx(v+bv) = ctx(v) + bv).
Softmax denominator comes from an appended ones-column on V and is applied
with a post-transpose `divide` on the Pool engine.
"""

import numpy as np

import concourse.bass as bass
import concourse.mybir as mybir
import concourse.tile as tile
from concourse import bacc
from concourse.masks import make_identity
from concourse.bass_utils import run_bass_kernel_spmd

F32 = mybir.dt.float32
F32R = mybir.dt.float32r
BF16 = mybir.dt.bfloat16
AX = mybir.AluOpType
ACT = mybir.ActivationFunctionType

B, S, H = 4, 2048, 1024
NH, HD = 16, 64
LN_EPS = 1e-12
N_CORES = 8
HPC = NH // 2          # 8 heads per core
WCOLS = 3 * HPC * HD   # 1536
TOKCH = S // 128       # 16 token chunks
SCALE = 1.0 / np.sqrt(HD)
QW = 512               # q columns per attention unit
NQQ = S // QW          # 4 q-quarters
CTX_DELAY = 24         # iterations between score matmul and ctx matmul
EXP_MOD = 5            # exp engine pattern period
EXP_ACT = 3            # iters per period routed to ACT (rest: DVE copy + Pool pow)
P2BUFS = 27            # P tile ring depth

_CACHE = {}


def _bcast(ap, n, axis=1):
    """Insert a stride-0 broadcast dim of size n at `axis` of an AP."""
    new = [list(p) for p in ap.ap]
    new.insert(axis, [0, n])
    return bass.AP(tensor=ap.tensor, offset=ap.offset, ap=new)


def _build_program():
    nc = bacc.Bacc("TRN2", target_bir_lowering=False)

    hid_d = nc.dram_tensor("hid", [S, H], BF16, kind="ExternalInput")
    w_d = nc.dram_tensor("w", [H, WCOLS], BF16, kind="ExternalInput")
    sin_d = nc.dram_tensor("sintab", [S, HD], F32, kind="ExternalInput")
    cos_d = nc.dram_tensor("costab", [S, HD], F32, kind="ExternalInput")
    bq_d = nc.dram_tensor("bq", [128, 512], F32, kind="ExternalInput")
    bk_d = nc.dram_tensor("bk", [128, 512], F32, kind="ExternalInput")
    bv_d = nc.dram_tensor("bv", [128, 512], F32, kind="ExternalInput")
    out_d = nc.dram_tensor("out", [S, HPC * HD], F32, kind="ExternalOutput")

    with tile.TileContext(nc) as tc:
        with tc.tile_pool(name="const", bufs=1) as const, \
             tc.tile_pool(name="store", bufs=1) as store:
            idb_s = const.tile([128, 128], BF16)
            make_identity(nc, idb_s)
            sin_s = const.tile([128, TOKCH, HD], F32)
            cos_s = const.tile([128, TOKCH, HD], F32)
            bq_s = const.tile([128, HPC, HD], F32)
            bk_s = const.tile([128, HPC, HD], F32)
            bv_s = const.tile([128, HPC * HD], F32)
            base_s = const.tile([128, QW], F32)
            nc.vector.memset(base_s, float(np.exp(SCALE)))
            eps_s = const.tile([128, 1], F32)
            nc.vector.memset(eps_s, LN_EPS)
            expwarm = const.tile([128, 1], F32)
            nc.scalar.activation(expwarm, eps_s, ACT.Exp)
            nhalf_s = const.tile([128, 1], F32)
            nc.vector.memset(nhalf_s, -0.5)

            # Transposed per-head q/k: head h lives at partitions (h%2)*64,
            # pair index h//2:  [128, 4, TOKCH, 128]  (= [64, tokch*128]/head)
            qT = store.tile([128, HPC // 2, TOKCH, 128], BF16)
            kT = store.tile([128, HPC // 2, TOKCH, 128], BF16)
            # v with appended ones column: [tok, chunk, head, 65]
            vA = store.tile([128, TOKCH, HPC, HD + 1], BF16)
            nc.vector.memset(vA[:, :, :, HD:HD + 1], 1.0)

            # ---------------- Phase 1: LN + QKV + RoPE + transposes ----------
            with tc.tile_pool(name="wpool", bufs=1) as wpool, \
                 tc.tile_pool(name="p1", bufs=2) as p1, \
                 tc.tile_pool(name="p1h", bufs=7) as p1h, \
                 tc.tile_pool(name="p1n", bufs=3) as p1n, \
                 tc.tile_pool(name="p1s", bufs=4) as p1s, \
                 tc.tile_pool(name="trq", bufs=4, space="PSUM") as trq, \
                 tc.tile_pool(name="qkvp", bufs=1, space="PSUM") as qkvp:
                w_r = w_d.rearrange("(a p) n -> p a n", p=128)
                w_s = []
                for hc in range(H // 128):
                    wt = wpool.tile([128, WCOLS], BF16, tag=f"w{hc}", name=f"w{hc}")
                    w_s.append(wt)
                ht_tiles = {}
                ht = p1h.tile([128, H], BF16, tag="ht")
                nc.sync.dma_start(out=ht, in_=hid_d[0:128, :])
                ht_tiles[0] = ht
                # PE p-state warmup while DMAs land (~3us of continuous busy)
                for wu in range(90):
                    ptw = trq.tile([128, 4, 128], BF16, tag="pt4",
                                   padded_shape=[128, 4, 256])
                    nc.tensor.transpose(ptw[:, 0, :], idb_s, idb_s)
                # w has priority: chunk 0's QKV needs the full weight load
                for hc in range(H // 128):
                    nc.sync.dma_start(out=w_s[hc], in_=w_r[:, hc, :])
                for tpre in range(1, 6):
                    htn = p1h.tile([128, H], BF16, tag="ht", name=f"ht{tpre}")
                    nc.sync.dma_start(out=htn, in_=hid_d[tpre * 128:(tpre + 1) * 128, :])
                    ht_tiles[tpre] = htn
                nc.sync.dma_start(out=bq_s.rearrange("p h d -> p (h d)"), in_=bq_d[:, :])
                nc.sync.dma_start(out=bk_s.rearrange("p h d -> p (h d)"), in_=bk_d[:, :])
                nc.sync.dma_start(out=sin_s, in_=sin_d.rearrange("(t p) d -> p t d", p=128))
                nc.sync.dma_start(out=cos_s, in_=cos_d.rearrange("(t p) d -> p t d", p=128))
                nc.sync.dma_start(out=bv_s, in_=bv_d[:, :])

                def rope_pre(t, pq):
                    # GPSIMD cannot touch PSUM: DVE does both fused
                    # bias-add + PSUM->SBUF copies, emitted early so the
                    # Pool-side rope chain can start promptly.
                    outs = []
                    for nch in range(2):
                        q0 = p1.tile([128, HPC, HD], F32, tag=f"q0{nch}", name="q0")
                        nc.vector.tensor_tensor(out=q0,
                                          in0=pq[nch].rearrange("p (h d) -> p h d", d=HD),
                                          in1=bq_s if nch == 0 else bk_s, op=AX.add)
                        outs.append(q0)
                    return outs

                def rope_block(t, q0k0):
                    sin_t = sin_s[:, t, :]
                    cos_t = cos_s[:, t, :]
                    for nch in range(2):
                        eng = nc.vector if nch == 0 else nc.gpsimd
                        q0 = q0k0[nch]
                        rp = p1.tile([128, HPC, HD], F32, tag=f"rp{nch}")
                        eng.tensor_tensor(out=rp[:, :, 0::2], in0=q0[:, :, 1::2],
                                          in1=_bcast(sin_t[:, 0::2], HPC), op=AX.mult)
                        eng.tensor_tensor(out=rp[:, :, 1::2], in0=q0[:, :, 0::2],
                                          in1=_bcast(sin_t[:, 1::2], HPC), op=AX.mult)
                        qf = p1.tile([128, HPC, HD], BF16, tag=f"qf{nch}")
                        eng.tensor_tensor(out=qf, in0=q0, in1=_bcast(cos_t, HPC),
                                          op=AX.mult)
                        eng.tensor_tensor(out=qf, in0=qf, in1=rp, op=AX.add)

                        dst = qT if nch == 0 else kT
                        pt4 = trq.tile([128, HPC // 2, 128], BF16, tag="pt4",
                                       padded_shape=[128, HPC // 2, 256])
                        qfv = qf.rearrange("p h d -> p (h d)")
                        for j in range(HPC // 2):
                            nc.tensor.transpose(pt4[:, j, :],
                                                qfv[:, j * 128:(j + 1) * 128], idb_s)
                        if nch == 0:
                            nc.vector.tensor_copy(dst[:, :, t, :], pt4)
                        else:
                            nc.scalar.copy(dst[:, :, t, :], pt4)

                pq_of = {}
                pre_of = {}
                hn_of = {}

                def ln_block(t):
                    ht = ht_tiles.pop(t)
                    st6 = p1s.tile([128, 2, 6], F32, tag="st6")
                    for half in range(2):
                        nc.vector.bn_stats(out=st6[:, half, :],
                                           in_=ht[:, half * 512:(half + 1) * 512])
                    mv = p1s.tile([128, 2], F32, tag="mv")
                    nc.vector.bn_aggr(out=mv, in_=st6)
                    vpe = p1s.tile([128, 1], F32, tag="vpe")
                    nc.gpsimd.tensor_scalar(out=vpe, in0=mv[:, 1:2], scalar1=LN_EPS,
                                            scalar2=None, op0=AX.add)
                    rstd = p1s.tile([128, 1], F32, tag="rstd")
                    nc.gpsimd.tensor_tensor(out=rstd, in0=vpe, in1=nhalf_s, op=AX.pow)
                    nmr = p1s.tile([128, 1], F32, tag="nmr")
                    nc.gpsimd.tensor_scalar(out=nmr, in0=mv[:, 0:1], scalar1=rstd,
                                            scalar2=-1.0, op0=AX.mult, op1=AX.mult)
                    hn = p1n.tile([128, H], BF16, tag="hn")
                    nc.scalar.activation(hn, ht, ACT.Identity, bias=nmr, scale=rstd)
                    return hn

                hn_of[0] = ln_block(0)
                for t in range(TOKCH):
                    if t + 6 < TOKCH:
                        htn = p1h.tile([128, H], BF16, tag="ht")
                        nc.sync.dma_start(out=htn, in_=hid_d[(t + 6) * 128:(t + 7) * 128, :])
                        ht_tiles[t + 6] = htn
                    # early: free last iterations' QKV PSUM slots
                    if t - 1 in pq_of:
                        nc.scalar.copy(vA[:, t - 1, :, 0:HD],
                                       pq_of[t - 1][2].rearrange("p (h d) -> p h d", d=HD))
                    if t - 2 in pq_of:
                        pre_of[t - 2] = rope_pre(t - 2, pq_of.pop(t - 2))
                    if t + 1 < TOKCH:
                        hn_of[t + 1] = ln_block(t + 1)
                    hn = hn_of.pop(t)

                    # transpose hn -> hT [hch, tok]
                    hT = p1.tile([128, H // 128, 128], BF16, tag="hT")
                    for g in range(2):
                        ptg = trq.tile([128, 4, 128], BF16, tag="pt4",
                                       padded_shape=[128, 4, 256])
                        for hc in range(4):
                            nc.tensor.transpose(ptg[:, hc, :],
                                                hn[:, (g * 4 + hc) * 128:(g * 4 + hc + 1) * 128],
                                                idb_s)
                        nc.vector.tensor_copy(hT[:, g * 4:(g + 1) * 4, :], ptg)

                    # QKV: out[tok, n] accumulated over h-chunks. Chunk 0 is
                    # paced by the weight DMA: go hc-major with keep-warm
                    # transposes stuffed between chunks so the PE p-state
                    # ramp survives the DMA waits.
                    pq = []
                    for nch in range(3):
                        bufs = 2 if nch == 0 else 1
                        pp = qkvp.tile([128, 512], F32, tag=f"qkv{nch}", bufs=bufs)
                        pq.append(pp)
                    if t == 0:
                        for hc in range(H // 128):
                            for nch in range(3):
                                nc.tensor.matmul(pq[nch], lhsT=hT[:, hc, :],
                                                 rhs=w_s[hc][:, nch * 512:(nch + 1) * 512],
                                                 start=(hc == 0), stop=(hc == H // 128 - 1))
                            if hc < H // 128 - 1:
                                for wu in range(8):
                                    ptw = trq.tile([128, 4, 128], BF16, tag="pt4",
                                                   padded_shape=[128, 4, 256])
                                    nc.tensor.transpose(ptw[:, 0, :], idb_s, idb_s)
                    else:
                        for nch in range(3):
                            for hc in range(H // 128):
                                nc.tensor.matmul(pq[nch], lhsT=hT[:, hc, :],
                                                 rhs=w_s[hc][:, nch * 512:(nch + 1) * 512],
                                                 start=(hc == 0), stop=(hc == H // 128 - 1))
                    pq_of[t] = pq

                    # tighten the tail: last chunk ropes at delay 0/1
                    if t == TOKCH - 1:
                        nc.scalar.copy(vA[:, t, :, 0:HD],
                                       pq[2].rearrange("p (h d) -> p h d", d=HD))
                        for t_ in (t - 1, t):
                            if t_ in pq_of:
                                pre_of[t_] = rope_pre(t_, pq_of.pop(t_))

                    # late: rope math + transposes for t-2
                    if t - 2 in pre_of:
                        rope_block(t - 2, pre_of.pop(t - 2))

                # drain remaining rope math
                for t_ in sorted(pre_of):
                    rope_block(t_, pre_of.pop(t_))

            # ---------------- Phase 2: attention per (head, q-quarter) -------
            with tc.tile_pool(name="p2", bufs=P2BUFS) as p2, \
                 tc.tile_pool(name="p2r", bufs=4) as p2r, \
                 tc.tile_pool(name="p2c", bufs=2) as p2c, \
                 tc.tile_pool(name="p2o", bufs=2) as p2o, \
                 tc.tile_pool(name="ctxp", bufs=2, space="PSUM") as ctxp, \
                 tc.tile_pool(name="ctxo", bufs=2, space="PSUM") as ctxo, \
                 tc.tile_pool(name="stp", bufs=4, space="PSUM") as stp:
                units = [(h, qq) for h in range(HPC) for qq in range(NQQ)]
                ctx_of = {}

                def emit_score(u, kc):
                    h, qq = u
                    po = (h % 2) * 64
                    pr = h // 2
                    sp = stp.tile([128, QW], F32, tag="st")
                    nc.tensor.matmul(
                        sp,
                        lhsT=kT[po:po + 64, pr, kc, :],
                        rhs=qT[po:po + 64, pr, qq * 4:(qq + 1) * 4, :],
                        start=True, stop=True)
                    return sp

                def emit_exp(i, sp):
                    # DVE/Pool have no exp; Pool has pow but cannot read
                    # PSUM. Route 3/5 through ACT directly and 2/5 through
                    # a DVE PSUM->SBUF copy followed by Pool pow(e^s, x).
                    P = p2.tile([128, QW], BF16, tag="P")
                    if i % EXP_MOD < EXP_ACT:
                        nc.scalar.activation(P, sp, ACT.Exp, scale=SCALE)
                    else:
                        spb = p2.tile([128, QW], F32, tag="spb", bufs=4)
                        nc.vector.tensor_copy(spb, sp)
                        nc.gpsimd.tensor_tensor(out=P, in0=base_s, in1=spb,
                                                op=AX.pow)
                    return P

                def emit_ctx(u, kc, P):
                    h, qq = u
                    if kc == 0:
                        ctx_of[u] = ctxp.tile([HD + 1, QW], F32, tag="ctx", name="cp")
                    cp = ctx_of[u]
                    nc.tensor.matmul(cp, lhsT=vA[:, kc, h, :], rhs=P,
                                     start=(kc == 0), stop=(kc == TOKCH - 1))
                    if kc == TOKCH - 1:
                        emit_tail(u, cp)

                def emit_tail(u, cp):
                    h, qq = u
                    ctxs = p2c.tile([HD + 1, QW], BF16, tag="ctxs")
                    nc.scalar.copy(ctxs, cp)
                    outt = p2o.tile([128, QW // 128, HD], F32, tag="outt")
                    for tc_ in range(QW // 128):
                        co = ctxo.tile([128, HD + 1], BF16, tag="co",
                                       padded_shape=[128, 1024])
                        nc.tensor.transpose(co, ctxs[:, tc_ * 128:(tc_ + 1) * 128],
                                            idb_s[0:HD + 1, 0:HD + 1])
                        rec = p2r.tile([128, 1], F32, tag="rec")
                        nc.vector.reciprocal(rec, co[:, HD:HD + 1])
                        nc.vector.tensor_scalar(out=outt[:, tc_, :], in0=co[:, 0:HD],
                                                scalar1=rec, scalar2=None,
                                                op0=AX.mult)
                    # fold in the v bias: ctx(v + bv) = ctx(v) + bv
                    nc.gpsimd.tensor_tensor(
                        out=outt, in0=outt,
                        in1=_bcast(bv_s[:, h * HD:(h + 1) * HD], QW // 128),
                        op=AX.add)
                    dst = out_d[qq * QW:(qq + 1) * QW, h * HD:(h + 1) * HD]
                    nc.sync.dma_start(
                        out=dst.rearrange("(c p) d -> p c d", p=128), in_=outt)

                stream = [(u, kc) for u in units for kc in range(TOKCH)]
                pend = []  # [(u, kc, P)]
                for i, (u, kc) in enumerate(stream):
                    sp = emit_score(u, kc)
                    P = emit_exp(i, sp)
                    pend.append((u, kc, P))
                    if len(pend) > CTX_DELAY:
                        emit_ctx(*pend.pop(0))
                for item in pend:
                    emit_ctx(*item)

    nc.compile()
    return nc


def _host_inputs(hidden_states, sinusoidal_pos, ln_weight, ln_bias, w_qkv, b_qkv):
    """Build the per-core input maps."""
    import ml_dtypes

    hidden_states = np.ascontiguousarray(hidden_states, dtype=np.float32)
    w_qkv = np.asarray(w_qkv, dtype=np.float32)
    b_qkv = np.asarray(b_qkv, dtype=np.float32)
    ln_weight = np.asarray(ln_weight, dtype=np.float32)
    ln_bias = np.asarray(ln_bias, dtype=np.float32)
    sp = np.asarray(sinusoidal_pos, dtype=np.float32).reshape(S, HD)

    # Fold LayerNorm affine params into the projection.
    w_eff = ln_weight[:, None] * w_qkv          # [H, 3H]
    b_eff = b_qkv + ln_bias @ w_qkv             # [3H]

    sin = sp[:, :HD // 2]
    cos = sp[:, HD // 2:]
    sin_pos = np.repeat(sin, 2, axis=1)          # [S, 64], col 2i = 2i+1 = sin_i
    cos_pos = np.repeat(cos, 2, axis=1)
    sgn = np.ones((1, HD), np.float32)
    sgn[0, 0::2] = -1.0
    sin_signed = (sin_pos * sgn).astype(np.float32)  # col 2i = -sin_i, 2i+1 = sin_i

    in_maps = []
    for c in range(N_CORES):
        b = c // 2
        g = c % 2
        cols = np.concatenate([
            np.arange(g * 512, (g + 1) * 512),
            1024 + np.arange(g * 512, (g + 1) * 512),
            2048 + np.arange(g * 512, (g + 1) * 512),
        ])
        bias = b_eff[cols]
        in_maps.append({
            "hid": hidden_states[b].astype(ml_dtypes.bfloat16),
            "w": np.ascontiguousarray(w_eff[:, cols]).astype(ml_dtypes.bfloat16),
            "bq": np.ascontiguousarray(
                np.broadcast_to(bias[None, 0:512], (128, 512))),
            "bk": np.ascontiguousarray(
                np.broadcast_to(bias[None, 512:1024], (128, 512))),
            "bv": np.ascontiguousarray(
                np.broadcast_to(bias[None, 1024:1536], (128, 512))),
            "sintab": sin_signed,
            "costab": cos_pos,
        })
    return in_maps


def _run(trace=False, **inputs):
    if "nc" not in _CACHE:
        _CACHE["nc"] = _build_program()
    nc = _CACHE["nc"]
    in_maps = _host_inputs(**inputs)
    res = run_bass_kernel_spmd(nc, in_maps, core_ids=list(range(N_CORES)),
                               trace=trace)
    out = np.empty((B, S, H), np.float32)
    for c in range(N_CORES):
        b = c // 2
        g = c % 2
        out[b, :, g * 512:(g + 1) * 512] = res.results[c]["out"]
    return out, res


def kernel(**inputs):
    out, _ = _run(trace=False, **inputs)
    return out


def kernel_traced(**inputs):
    return _run(trace=True, **inputs)


# revision 28
# speedup vs baseline: 1.0079x; 1.0010x over previous
"""RoFormer self-attention (LN + fused QKV + interleaved RoPE + SDPA) on 8 trn2 cores.

Sharding: core c -> batch b = c//2, head-group g = c%2 (8 of 16 heads).
Each core computes LN + QKV for its batch, RoPE, and full non-causal
attention for its 8 heads, writing the [2048, 512] slice
out[b, :, 512g:512(g+1)]. No collectives.

Numerics: all matmuls in bf16 with fp32 PSUM accumulation (rel err ~4e-3).
Softmax skips max-subtraction; exp work is split between the Activation
engine (native Exp) and DVE (pow(e^scale, x) with a broadcast base).
QKV q/k biases are added on DVE/Pool as fused bias+copy ops out of PSUM;
the v bias is folded into the context output (ctx(v+bv) = ctx(v) + bv).
Softmax denominator comes from an appended ones-column on V and is applied
with a post-transpose `divide` on the Pool engine.
"""

import numpy as np

import concourse.bass as bass
import concourse.mybir as mybir
import concourse.tile as tile
from concourse import bacc
from concourse.masks import make_identity
from concourse.bass_utils import run_bass_kernel_spmd

F32 = mybir.dt.float32
F32R = mybir.dt.float32r
BF16 = mybir.dt.bfloat16
AX = mybir.AluOpType
ACT = mybir.ActivationFunctionType

B, S, H = 4, 2048, 1024
NH, HD = 16, 64
LN_EPS = 1e-12
N_CORES = 8
HPC = NH // 2          # 8 heads per core
WCOLS = 3 * HPC * HD   # 1536
TOKCH = S // 128       # 16 token chunks
SCALE = 1.0 / np.sqrt(HD)
QW = 512               # q columns per attention unit
NQQ = S // QW          # 4 q-quarters
CTX_DELAY = 24         # iterations between score matmul and ctx matmul
EXP_MOD = 5            # exp engine pattern period
EXP_ACT = 3            # iters per period routed to ACT (rest: DVE copy + Pool pow)
P2BUFS = 27            # P tile ring depth

_CACHE = {}


def _bcast(ap, n, axis=1):
    """Insert a stride-0 broadcast dim of size n at `axis` of an AP."""
    new = [list(p) for p in ap.ap]
    new.insert(axis, [0, n])
    return bass.AP(tensor=ap.tensor, offset=ap.offset, ap=new)


def _build_program():
    nc = bacc.Bacc("TRN2", target_bir_lowering=False)

    hid_d = nc.dram_tensor("hid", [S, H], BF16, kind="ExternalInput")
    w_d = nc.dram_tensor("w", [H, WCOLS], BF16, kind="ExternalInput")
    sin_d = nc.dram_tensor("sintab", [S, HD], F32, kind="ExternalInput")
    cos_d = nc.dram_tensor("costab", [S, HD], F32, kind="ExternalInput")
    bq_d = nc.dram_tensor("bq", [128, 512], F32, kind="ExternalInput")
    bk_d = nc.dram_tensor("bk", [128, 512], F32, kind="ExternalInput")
    bv_d = nc.dram_tensor("bv", [128, 512], F32, kind="ExternalInput")
    out_d = nc.dram_tensor("out", [S, HPC * HD], F32, kind="ExternalOutput")

    with tile.TileContext(nc) as tc:
        with tc.tile_pool(name="const", bufs=1) as const, \
             tc.tile_pool(name="store", bufs=1) as store:
            idb_s = const.tile([128, 128], BF16)
            make_identity(nc, idb_s)
            sin_s = const.tile([128, TOKCH, HD], F32)
            cos_s = const.tile([128, TOKCH, HD], F32)
            bq_s = const.tile([128, HPC, HD], F32)
            bk_s = const.tile([128, HPC, HD], F32)
            bv_s = const.tile([128, HPC * HD], F32)
            base_s = const.tile([128, QW], F32)
            nc.vector.memset(base_s, float(np.exp(SCALE)))
            eps_s = const.tile([128, 1], F32)
            nc.vector.memset(eps_s, LN_EPS)
            expwarm = const.tile([128, 1], F32)
            nc.scalar.activation(expwarm, eps_s, ACT.Exp)
            nhalf_s = const.tile([128, 1], F32)
            nc.vector.memset(nhalf_s, -0.5)

            # Transposed per-head q/k: head h lives at partitions (h%2)*64,
            # pair index h//2:  [128, 4, TOKCH, 128]  (= [64, tokch*128]/head)
            qT = store.tile([128, HPC // 2, TOKCH, 128], BF16)
            kT = store.tile([128, HPC // 2, TOKCH, 128], BF16)
            # v with appended ones column: [tok, chunk, head, 65]
            vA = store.tile([128, TOKCH, HPC, HD + 1], BF16)
            nc.vector.memset(vA[:, :, :, HD:HD + 1], 1.0)

            # ---------------- Phase 1: LN + QKV + RoPE + transposes ----------
            with tc.tile_pool(name="wpool", bufs=1) as wpool, \
                 tc.tile_pool(name="p1", bufs=2) as p1, \
                 tc.tile_pool(name="p1h", bufs=7) as p1h, \
                 tc.tile_pool(name="p1n", bufs=3) as p1n, \
                 tc.tile_pool(name="p1s", bufs=4) as p1s, \
                 tc.tile_pool(name="trq", bufs=4, space="PSUM") as trq, \
                 tc.tile_pool(name="qkvp", bufs=1, space="PSUM") as qkvp:
                w_r = w_d.rearrange("(a p) n -> p a n", p=128)
                w_s = []
                for hc in range(H // 128):
                    wt = wpool.tile([128, WCOLS], BF16, tag=f"w{hc}", name=f"w{hc}")
                    w_s.append(wt)
                ht_tiles = {}
                ht = p1h.tile([128, H], BF16, tag="ht")
                nc.sync.dma_start(out=ht, in_=hid_d[0:128, :])
                ht_tiles[0] = ht
                # PE p-state warmup while DMAs land (~3us of continuous busy)
                for wu in range(90):
                    ptw = trq.tile([128, 4, 128], BF16, tag="pt4",
                                   padded_shape=[128, 4, 256])
                    nc.tensor.transpose(ptw[:, 0, :], idb_s, idb_s)
                # w has priority: chunk 0's QKV needs the full weight load
                for hc in range(H // 128):
                    nc.sync.dma_start(out=w_s[hc], in_=w_r[:, hc, :])
                for tpre in range(1, 6):
                    htn = p1h.tile([128, H], BF16, tag="ht", name=f"ht{tpre}")
                    nc.sync.dma_start(out=htn, in_=hid_d[tpre * 128:(tpre + 1) * 128, :])
                    ht_tiles[tpre] = htn
                nc.sync.dma_start(out=bq_s.rearrange("p h d -> p (h d)"), in_=bq_d[:, :])
                nc.sync.dma_start(out=bk_s.rearrange("p h d -> p (h d)"), in_=bk_d[:, :])
                nc.sync.dma_start(out=sin_s, in_=sin_d.rearrange("(t p) d -> p t d", p=128))
                nc.sync.dma_start(out=cos_s, in_=cos_d.rearrange("(t p) d -> p t d", p=128))
                nc.sync.dma_start(out=bv_s, in_=bv_d[:, :])

                def rope_pre(t, pq):
                    # GPSIMD cannot touch PSUM: DVE does both fused
                    # bias-add + PSUM->SBUF copies, emitted early so the
                    # Pool-side rope chain can start promptly.
                    outs = []
                    for nch in range(2):
                        q0 = p1.tile([128, HPC, HD], F32, tag=f"q0{nch}", name="q0")
                        nc.vector.tensor_tensor(out=q0,
                                          in0=pq[nch].rearrange("p (h d) -> p h d", d=HD),
                                          in1=bq_s if nch == 0 else bk_s, op=AX.add)
                        outs.append(q0)
                    return outs

                def rope_block(t, q0k0):
                    sin_t = sin_s[:, t, :]
                    cos_t = cos_s[:, t, :]
                    for nch in range(2):
                        eng = nc.vector if nch == 0 else nc.gpsimd
                        q0 = q0k0[nch]
                        rp = p1.tile([128, HPC, HD], F32, tag=f"rp{nch}")
                        eng.tensor_tensor(out=rp[:, :, 0::2], in0=q0[:, :, 1::2],
                                          in1=_bcast(sin_t[:, 0::2], HPC), op=AX.mult)
                        eng.tensor_tensor(out=rp[:, :, 1::2], in0=q0[:, :, 0::2],
                                          in1=_bcast(sin_t[:, 1::2], HPC), op=AX.mult)
                        qf = p1.tile([128, HPC, HD], BF16, tag=f"qf{nch}")
                        eng.tensor_tensor(out=qf, in0=q0, in1=_bcast(cos_t, HPC),
                                          op=AX.mult)
                        eng.tensor_tensor(out=qf, in0=qf, in1=rp, op=AX.add)

                        dst = qT if nch == 0 else kT
                        pt4 = trq.tile([128, HPC // 2, 128], BF16, tag="pt4",
                                       padded_shape=[128, HPC // 2, 256])
                        qfv = qf.rearrange("p h d -> p (h d)")
                        for j in range(HPC // 2):
                            nc.tensor.transpose(pt4[:, j, :],
                                                qfv[:, j * 128:(j + 1) * 128], idb_s)
                        if nch == 0:
                            nc.vector.tensor_copy(dst[:, :, t, :], pt4)
                        else:
                            nc.scalar.copy(dst[:, :, t, :], pt4)

                pq_of = {}
                pre_of = {}
                hn_of = {}

                def ln_block(t):
                    ht = ht_tiles.pop(t)
                    st6 = p1s.tile([128, 2, 6], F32, tag="st6")
                    for half in range(2):
                        nc.vector.bn_stats(out=st6[:, half, :],
                                           in_=ht[:, half * 512:(half + 1) * 512])
                    mv = p1s.tile([128, 2], F32, tag="mv")
                    nc.vector.bn_aggr(out=mv, in_=st6)
                    vpe = p1s.tile([128, 1], F32, tag="vpe")
                    nc.gpsimd.tensor_scalar(out=vpe, in0=mv[:, 1:2], scalar1=LN_EPS,
                                            scalar2=None, op0=AX.add)
                    rstd = p1s.tile([128, 1], F32, tag="rstd")
                    nc.gpsimd.tensor_tensor(out=rstd, in0=vpe, in1=nhalf_s, op=AX.pow)
                    nmr = p1s.tile([128, 1], F32, tag="nmr")
                    nc.gpsimd.tensor_scalar(out=nmr, in0=mv[:, 0:1], scalar1=rstd,
                                            scalar2=-1.0, op0=AX.mult, op1=AX.mult)
                    hn = p1n.tile([128, H], BF16, tag="hn")
                    nc.scalar.activation(hn, ht, ACT.Identity, bias=nmr, scale=rstd)
                    return hn

                hn_of[0] = ln_block(0)
                for t in range(TOKCH):
                    if t + 6 < TOKCH:
                        htn = p1h.tile([128, H], BF16, tag="ht")
                        nc.sync.dma_start(out=htn, in_=hid_d[(t + 6) * 128:(t + 7) * 128, :])
                        ht_tiles[t + 6] = htn
                    # early: free last iterations' QKV PSUM slots
                    if t - 1 in pq_of:
                        nc.scalar.copy(vA[:, t - 1, :, 0:HD],
                                       pq_of[t - 1][2].rearrange("p (h d) -> p h d", d=HD))
                    if t - 2 in pq_of:
                        pre_of[t - 2] = rope_pre(t - 2, pq_of.pop(t - 2))
                    if t + 1 < TOKCH:
                        hn_of[t + 1] = ln_block(t + 1)
                    hn = hn_of.pop(t)

                    # transpose hn -> hT [hch, tok]
                    hT = p1.tile([128, H // 128, 128], BF16, tag="hT")
                    for g in range(2):
                        ptg = trq.tile([128, 4, 128], BF16, tag="pt4",
                                       padded_shape=[128, 4, 256])
                        for hc in range(4):
                            nc.tensor.transpose(ptg[:, hc, :],
                                                hn[:, (g * 4 + hc) * 128:(g * 4 + hc + 1) * 128],
                                                idb_s)
                        nc.vector.tensor_copy(hT[:, g * 4:(g + 1) * 4, :], ptg)

                    # QKV: out[tok, n] accumulated over h-chunks. Chunk 0 is
                    # paced by the weight DMA: go hc-major with keep-warm
                    # transposes stuffed between chunks so the PE p-state
                    # ramp survives the DMA waits.
                    pq = []
                    for nch in range(3):
                        bufs = 2 if nch == 0 else 1
                        pp = qkvp.tile([128, 512], F32, tag=f"qkv{nch}", bufs=bufs)
                        pq.append(pp)
                    if t == 0:
                        for hc in range(H // 128):
                            for nch in range(3):
                                nc.tensor.matmul(pq[nch], lhsT=hT[:, hc, :],
                                                 rhs=w_s[hc][:, nch * 512:(nch + 1) * 512],
                                                 start=(hc == 0), stop=(hc == H // 128 - 1))
                            if hc < H // 128 - 1:
                                for wu in range(8):
                                    ptw = trq.tile([128, 4, 128], BF16, tag="pt4",
                                                   padded_shape=[128, 4, 256])
                                    nc.tensor.transpose(ptw[:, 0, :], idb_s, idb_s)
                    else:
                        for nch in range(3):
                            for hc in range(H // 128):
                                nc.tensor.matmul(pq[nch], lhsT=hT[:, hc, :],
                                                 rhs=w_s[hc][:, nch * 512:(nch + 1) * 512],
                                                 start=(hc == 0), stop=(hc == H // 128 - 1))
                    pq_of[t] = pq

                    # tighten the tail: last chunk ropes at delay 0/1
                    if t == TOKCH - 1:
                        nc.scalar.copy(vA[:, t, :, 0:HD],
                                       pq[2].rearrange("p (h d) -> p h d", d=HD))
                        for t_ in (t - 1, t):
                            if t_ in pq_of:
                                pre_of[t_] = rope_pre(t_, pq_of.pop(t_))

                    # late: rope math + transposes for t-2
                    if t - 2 in pre_of:
                        rope_block(t - 2, pre_of.pop(t - 2))

                # drain remaining rope math
                for t_ in sorted(pre_of):
                    rope_block(t_, pre_of.pop(t_))

            # ---------------- Phase 2: attention per (head, q-quarter) -------
            with tc.tile_pool(name="p2", bufs=P2BUFS) as p2, \
                 tc.tile_pool(name="p2r", bufs=4) as p2r, \
                 tc.tile_pool(name="p2c", bufs=2) as p2c, \
                 tc.tile_pool(name="p2o", bufs=2) as p2o, \
                 tc.tile_pool(name="ctxp", bufs=2, space="PSUM") as ctxp, \
                 tc.tile_pool(name="ctxo", bufs=2, space="PSUM") as ctxo, \
                 tc.tile_pool(name="stp", bufs=4, space="PSUM") as stp:
                units = [(h, qq) for h in range(HPC) for qq in range(NQQ)]
                ctx_of = {}

                def emit_score(u, kc):
                    h, qq = u
                    po = (h % 2) * 64
                    pr = h // 2
                    sp = stp.tile([128, QW], F32, tag="st")
                    nc.tensor.matmul(
                        sp,
                        lhsT=kT[po:po + 64, pr, kc, :],
                        rhs=qT[po:po + 64, pr, qq * 4:(qq + 1) * 4, :],
                        start=True, stop=True)
                    return sp

                def emit_exp(i, sp):
                    # DVE/Pool have no exp; Pool has pow but cannot read
                    # PSUM. Route 3/5 through ACT directly and 2/5 through
                    # a DVE PSUM->SBUF copy followed by Pool pow(e^s, x).
                    P = p2.tile([128, QW], BF16, tag="P")
                    if i % EXP_MOD < EXP_ACT:
                        nc.scalar.activation(P, sp, ACT.Exp, scale=SCALE)
                    else:
                        spb = p2.tile([128, QW], F32, tag="spb", bufs=4)
                        nc.vector.tensor_copy(spb, sp)
                        nc.gpsimd.tensor_tensor(out=P, in0=base_s, in1=spb,
                                                op=AX.pow)
                    return P

                def emit_ctx(u, kc, P):
                    h, qq = u
                    if kc == 0:
                        ctx_of[u] = ctxp.tile([HD + 1, QW], F32, tag="ctx", name="cp")
                    cp = ctx_of[u]
                    nc.tensor.matmul(cp, lhsT=vA[:, kc, h, :], rhs=P,
                                     start=(kc == 0), stop=(kc == TOKCH - 1))
                    if kc == TOKCH - 1:
                        emit_tail(u, cp)

                def emit_tail(u, cp):
                    h, qq = u
                    ctxs = p2c.tile([HD + 1, QW], BF16, tag="ctxs")
                    nc.scalar.copy(ctxs, cp)
                    outt = p2o.tile([128, QW // 128, HD], F32, tag="outt")
                    for tc_ in range(QW // 128):
                        co = ctxo.tile([128, HD + 1], BF16, tag="co",
                                       padded_shape=[128, 1024])
                        nc.tensor.transpose(co, ctxs[:, tc_ * 128:(tc_ + 1) * 128],
                                            idb_s[0:HD + 1, 0:HD + 1])
                        rec = p2r.tile([128, 1], F32, tag="rec")
                        nc.vector.reciprocal(rec, co[:, HD:HD + 1])
                        nc.vector.tensor_scalar(out=outt[:, tc_, :], in0=co[:, 0:HD],
                                                scalar1=rec, scalar2=None,
                                                op0=AX.mult)
                    # fold in the v bias: ctx(v + bv) = ctx(v) + bv
                    nc.vector.tensor_tensor(
                        out=outt, in0=outt,
                        in1=_bcast(bv_s[:, h * HD:(h + 1) * HD], QW // 128),
                        op=AX.add)
                    dst = out_d[qq * QW:(qq + 1) * QW, h * HD:(h + 1) * HD]
                    nc.sync.dma_start(
                        out=dst.rearrange("(c p) d -> p c d", p=128), in_=outt)

                stream = [(u, kc) for u in units for kc in range(TOKCH)]
                pend = []  # [(u, kc, P)]
                for i, (u, kc) in enumerate(stream):
                    sp = emit_score(u, kc)
                    P = emit_exp(i, sp)
                    pend.append((u, kc, P))
                    if len(pend) > CTX_DELAY:
                        emit_ctx(*pend.pop(0))
                for item in pend:
                    emit_ctx(*item)

    nc.compile()
    return nc


def _host_inputs(hidden_states, sinusoidal_pos, ln_weight, ln_bias, w_qkv, b_qkv):
    """Build the per-core input maps."""
    import ml_dtypes

    hidden_states = np.ascontiguousarray(hidden_states, dtype=np.float32)
    w_qkv = np.asarray(w_qkv, dtype=np.float32)
    b_qkv = np.asarray(b_qkv, dtype=np.float32)
    ln_weight = np.asarray(ln_weight, dtype=np.float32)
    ln_bias = np.asarray(ln_bias, dtype=np.float32)
    sp = np.asarray(sinusoidal_pos, dtype=np.float32).reshape(S, HD)

    # Fold LayerNorm affine params into the projection.
    w_eff = ln_weight[:, None] * w_qkv          # [H, 3H]
    b_eff = b_qkv + ln_bias @ w_qkv             # [3H]

    sin = sp[:, :HD // 2]
    cos = sp[:, HD // 2:]
    sin_pos = np.repeat(sin, 2, axis=1)          # [S, 64], col 2i = 2i+1 = sin_i
    cos_pos = np.repeat(cos, 2, axis=1)
    sgn = np.ones((1, HD), np.float32)
    sgn[0, 0::2] = -1.0
    sin_signed = (sin_pos * sgn).astype(np.float32)  # col 2i = -sin_i, 2i+1 = sin_i

    in_maps = []
    for c in range(N_CORES):
        b = c // 2
        g = c % 2
        cols = np.concatenate([
            np.arange(g * 512, (g + 1) * 512),
            1024 + np.arange(g * 512, (g + 1) * 512),
            2048 + np.arange(g * 512, (g + 1) * 512),
        ])
        bias = b_eff[cols]
        in_maps.append({
            "hid": hidden_states[b].astype(ml_dtypes.bfloat16),
            "w": np.ascontiguousarray(w_eff[:, cols]).astype(ml_dtypes.bfloat16),
            "bq": np.ascontiguousarray(
                np.broadcast_to(bias[None, 0:512], (128, 512))),
            "bk": np.ascontiguousarray(
                np.broadcast_to(bias[None, 512:1024], (128, 512))),
            "bv": np.ascontiguousarray(
                np.broadcast_to(bias[None, 1024:1536], (128, 512))),
            "sintab": sin_signed,
            "costab": cos_pos,
        })
    return in_maps


def _run(trace=False, **inputs):
    if "nc" not in _CACHE:
        _CACHE["nc"] = _build_program()
    nc = _CACHE["nc"]
    in_maps = _host_inputs(**inputs)
    res = run_bass_kernel_spmd(nc, in_maps, core_ids=list(range(N_CORES)),
                               trace=trace)
    out = np.empty((B, S, H), np.float32)
    for c in range(N_CORES):
        b = c // 2
        g = c % 2
        out[b, :, g * 512:(g + 1) * 512] = res.results[c]["out"]
    return out, res


def kernel(**inputs):
    out, _ = _run(trace=False, **inputs)
    return out


def kernel_traced(**inputs):
    return _run(trace=True, **inputs)
